# revision 32
# baseline (speedup 1.0000x reference)
"""DeepseekV2 MoE layer on 8 Trainium2 NeuronCores (Bass/Tile, SPMD).

Strategy (expert-parallel, all-bf16 matmuls, fp32 PSUM accumulate):
 - Host computes the MoE gate routing in numpy (matches the jax reference:
   top-k margins are ~1e-4, far above ulp noise).
 - 16 experts -> 8 cores x 3 slots.  Slot capacities (C0>=C1>=C2) chosen
   by a small DP so expert token loads pack into the 24 slots with
   minimal total capacity; overfull experts are SPLIT across slots on
   different cores.  SPMD cores run an identical instruction stream, so
   per-core cost is C0+C1+C2 regardless of data.
 - Phase order: shared expert FIRST (its weight demand is only ~77 GB/s,
   so it absorbs the slow cold-start DMA window and gives the weight
   rings 60us of runway), then routed slots in descending cap; the small
   slot (which would need 408+ GB/s live) runs last, fully prefetched.
 - DMA scheduling: queue A (sync, HWDGE) carries every weight stream +
   x^T as ONE in-order deadline-sorted program of large partition-major
   transfers (>=4KB/partition line; small-descriptor DMAs run at ~1/3
   bandwidth).  In-order execution means a byte is never spent on a
   later-deadline transfer while an earlier one waits, and tile-ring WAR
   deps throttle run-ahead to the SBUF budget; each phase's first
   ring-depth of tiles is emitted from hook points in earlier phases.
   Queue B (gpsimd, SWDGE) carries gathered-x/cw/wsd prefetches behind a
   dummy "gate" read of an early activation tile so they cannot race the
   startup stream.  Outputs go on the scalar queue (with the silus),
   batched 8 d-tiles per DMA.  This removes the DGE descriptor
   contention and cross-queue bandwidth races of earlier versions.
 - Per slot: GEMM1 (wgu tiles stationary, gathered x^T moving) -> SiLU
   on ScalarE -> *up *cw on VectorE -> GEMM2 *transposed* (w_down^T
   stationary, activations moving) producing y^T [D, cap].  The routed
   gate weight (incl. the 2.5 scale) is applied via a host-replicated
   [128, cap] row.
 - Shared expert: TP-sharded over its intermediate dim (352/core, padded
   to 3x128), same pipeline; PSUM->SBUF copies alternate Vector/Scalar
   (vector alone rate-matches the tensor engine there and stutters).
 - bf16 operands halve HBM traffic vs fp32; measured rel err ~4e-3.
 - Host scatter-adds per-piece outputs and sums shared partials.
"""

import itertools
import numpy as np
import ml_dtypes
from contextlib import ExitStack

import concourse.bacc as bacc
import concourse.tile as tile
import concourse.mybir as mybir
from concourse.bass_utils import run_bass_kernel_spmd

# problem dims (fixed by the graded problem)
T, D, I, E = 1024, 2048, 1408, 16
SI = 2 * I               # shared expert intermediate (2816)
TOP_K, N_GROUP, TOPK_GROUP = 6, 4, 2
ROUTED_SCALE = 2.5
NCORES = 8
KT = D // 128            # 16 contraction tiles (gemm1)
IT = I // 128            # 11 intermediate tiles (routed)
DT = D // 128            # 16 output d-tiles (gemm2)
SSLICE = SI // NCORES    # 352 shared-intermediate rows per core
SIP = 384                # padded to 3x128
SIT = SIP // 128         # 3

f32 = mybir.dt.float32
bf16 = mybir.dt.bfloat16
BF = ml_dtypes.bfloat16
ACT_SILU = mybir.ActivationFunctionType.Silu
ACT_COPY = mybir.ActivationFunctionType.Copy

WGU_BUFS = 6             # wgu ring depth (cross-phase weight prefetch)
WD_BUFS = 6              # wd ring depth (2 d-tiles per buf)
OG = 8                   # d-tiles per output staging group (routed)
OGS = 2                  # d-tiles per output staging group (shared)


# ---------------------------------------------------------------- routing
def _route(x, gate_w, bias):
    """Replicates the jax reference gate in numpy f32 (decision margins are
    >=1e-4 so ulp-level differences cannot flip the top-k).

    Returns topk_idx [T,6] int, weights [T,6] f32 (renormalized, unscaled).
    """
    logits = (x @ gate_w.T).astype(np.float32)
    scores = (1.0 / (1.0 + np.exp(-logits))).astype(np.float32)
    s_choice = scores + bias.astype(np.float32)
    grp = s_choice.reshape(T, N_GROUP, E // N_GROUP)
    group_scores = np.sort(grp, axis=2)[:, :, -2:].sum(2, dtype=np.float32)
    grp_idx = np.argsort(-group_scores, axis=1, kind="stable")[:, :TOPK_GROUP]
    gmask = np.zeros((T, N_GROUP), dtype=bool)
    gmask[np.arange(T)[:, None], grp_idx] = True
    emask = np.repeat(gmask, E // N_GROUP, axis=1)
    masked = np.where(emask, s_choice, -np.inf)
    topk_idx = np.argsort(-masked, axis=1, kind="stable")[:, :TOP_K]
    w = np.take_along_axis(scores, topk_idx, axis=1)
    w = (w / w.sum(axis=1, keepdims=True)).astype(np.float32)
    return topk_idx, w


# --------------------------------------------------------- slot assignment
def _feasible(caps, counts, max_pieces=3):
    """Can each expert be covered by <=max_pieces slots (8 per capacity
    class) with total capacity >= its token count?  Returns per-expert
    class-use tuples or None."""
    nclass = len(caps)
    opts_per_expert = []
    for cnt in counts:
        opts = []
        for a in itertools.product(range(max_pieces + 1), repeat=nclass):
            if sum(a) == 0 or sum(a) > max_pieces:
                continue
            tot = sum(ai * c for ai, c in zip(a, caps))
            if tot >= cnt:
                dom = any(a[j] > 0 and tot - caps[j] >= cnt for j in range(nclass))
                if not dom:
                    opts.append(a)
        if not opts:
            return None
        opts_per_expert.append(opts)
    states = {(0,) * nclass: []}
    for opts in opts_per_expert:
        new = {}
        for st, hist in states.items():
            for a in opts:
                nst = tuple(s + ai for s, ai in zip(st, a))
                if all(n <= NCORES for n in nst) and nst not in new:
                    new[nst] = hist + [a]
        if not new:
            return None
        states = new
    return next(iter(states.values()))


def _solve_slots(counts):
    """Pick 3 slot capacities (multiples of 8, each <=512) minimizing
    C0+C1+C2 s.t. the expert loads pack into 8 slots per class.
    Returns (caps, per-expert class-use tuples)."""
    total = int(counts.sum())
    lo = -(-total // NCORES)
    lo = -(-lo // 8) * 8
    for s in range(lo, 1537, 8):
        for C0 in range(-(-s // 3) // 8 * 8, min(512, s - 16) + 1, 8):
            for C1 in range(8, min(C0, s - C0 - 8) + 1, 8):
                C2 = s - C0 - C1
                if C2 < 8 or C2 > C1:
                    continue
                r = _feasible((C0, C1, C2), counts)
                if r is not None:
                    return [C0, C1, C2], r
    raise RuntimeError("no feasible slot packing")


# ------------------------------------------------------------ host packing
def _pack_wgu(w2, it_cnt):
    """w2: [2*ic, D] rows (gate block then up block, ic=128*it_cnt rows)
    -> [it_cnt, 128, 2, KT, 128] bf16: tile (t, j) is wgu^T laid out so
    [p, c] = w2[j_block + t*128 + c, k*128 + p] (lhsT k-slices)."""
    ic = 128 * it_cnt
    g = w2[:ic].reshape(it_cnt, 128, D)
    u = w2[ic:].reshape(it_cnt, 128, D)
    arr = np.stack([g, u], axis=1)               # (t, j, c, D)
    arr = arr.reshape(it_cnt, 2, 128, KT, 128)   # (t, j, c, k, p)
    return np.ascontiguousarray(arr.transpose(0, 4, 1, 3, 2), dtype=BF)


def _pack_wdT(wd, it_cnt, grp):
    """wd: [D, 128*it_cnt] (w_down, cols may be zero padded)
    -> [DT//grp, 128, grp, it_cnt, 128] bf16: tile (g)[:, e, k, :] is the
    lhsT for d-tile dt=g*grp+e: [i, d] = wd[dt*128 + d, k*128 + i]."""
    a = wd.reshape(DT, 128, it_cnt, 128)         # (dt, d, k, i)
    a = a.transpose(0, 3, 2, 1)                  # (dt, i, k, d)
    a = a.reshape(DT // grp, grp, 128, it_cnt, 128)
    return np.ascontiguousarray(a.transpose(0, 2, 1, 3, 4), dtype=BF)


def _pack_xT(xs, cap):
    """xs: [n, D] token rows -> [128, KT, cap] bf16 (x^T k-tiles, padded)."""
    out = np.zeros((128, KT, cap), dtype=BF)
    n = xs.shape[0]
    if n:
        out[:, :, :n] = xs.T.reshape(KT, 128, n).transpose(1, 0, 2).astype(BF)
    return out


def _unpack_y(arr, cap):
    """[DT//og, 128, og, cap] bf16 device output -> [D, cap] f32."""
    return arr.transpose(0, 2, 1, 3).reshape(D, cap).astype(np.float32)


# ------------------------------------------------------------ device build
def _build(caps):
    nc = bacc.Bacc("TRN2", target_bir_lowering=False, debug=False,
                   num_devices=NCORES)
    S = len(caps)
    xg_d = [nc.dram_tensor(f"xg{s}", [128, KT, caps[s]], bf16, kind="ExternalInput") for s in range(S)]
    cw_d = [nc.dram_tensor(f"cw{s}", [128, caps[s]], f32, kind="ExternalInput") for s in range(S)]
    wgu_d = [nc.dram_tensor(f"wgu{s}", [IT, 128, 2, KT, 128], bf16, kind="ExternalInput") for s in range(S)]
    wd_d = [nc.dram_tensor(f"wd{s}", [DT // 2, 128, 2, IT, 128], bf16, kind="ExternalInput") for s in range(S)]
    yr_d = [nc.dram_tensor(f"yr{s}", [DT // OG, 128, OG, caps[s]], bf16, kind="ExternalOutput") for s in range(S)]
    xt_d = nc.dram_tensor("xt", [128, KT, T], bf16, kind="ExternalInput")
    wsgu_d = nc.dram_tensor("wsgu", [SIT, 128, 2, KT, 128], bf16, kind="ExternalInput")
    wsd_d = nc.dram_tensor("wsd", [128, DT, SIT, 128], bf16, kind="ExternalInput")
    ys_d = nc.dram_tensor("ys", [DT // OGS, 128, OGS, T], bf16, kind="ExternalOutput")
    dum_d = nc.dram_tensor("dum", [1, 8], bf16, kind="Internal")

    with tile.TileContext(nc) as tc, ExitStack() as ctx:
        sb = ctx.enter_context(tc.tile_pool(name="sb", bufs=1))
        ps = ctx.enter_context(tc.tile_pool(name="ps", bufs=1, space="PSUM"))

        q = KT // 4
        xgs = [sb.tile([128, KT, caps[s]], bf16, tag=f"xg{s}", bufs=1,
                       name=f"xga{s}") for s in range(S)]
        cws = [sb.tile([128, caps[s]], f32, tag=f"cw{s}", bufs=1,
                       name=f"cwa{s}") for s in range(S)]
        xt = sb.tile([128, KT, T], bf16, tag="xt", bufs=1, name="xt")
        wsd = sb.tile([128, DT, SIT, 128], bf16, tag="wsd", bufs=1, name="wsd")
        # phase order: shared, slot0 (344), slot1 (280), slot2 (192)
        ats = [sb.tile([128, SIT, T], bf16, tag="at", bufs=2, name="atsh"),
               sb.tile([128, IT, caps[0]], bf16, tag="at", bufs=2, name="at0"),
               sb.tile([128, IT, caps[1]], bf16, tag="at", bufs=2, name="at1"),
               sb.tile([128, IT, caps[2]], bf16, tag="at", bufs=2, name="at2")]

        # ---- queue A (sync, HWDGE): all weight streams + x^T, one in-order
        # deadline-sorted program.  In-order execution means a byte is never
        # spent on a later-deadline transfer while an earlier-deadline one
        # waits; ring WAR deps throttle run-ahead to the SBUF budget.  Each
        # phase's first ring-depth of tiles is emitted from hook points in
        # earlier phases (prefetch), the rest from its own loops.  All
        # transfers are >=4KB per partition line - small-descriptor DMAs run
        # at ~1/3 bandwidth and strangle the in-order queue.
        wgu_lists = [[], [], [], []]
        wd_lists = [None, [], [], []]
        wgu_drams = [wsgu_d, wgu_d[0], wgu_d[1], wgu_d[2]]
        wd_drams = [None, wd_d[0], wd_d[1], wd_d[2]]

        def mk_wgu(p, ts):
            dram = wgu_drams[p]
            for t in ts:
                w = sb.tile([128, 2, KT, 128], bf16, tag="wgu",
                            bufs=WGU_BUFS, name="wgu")
                if p == 0 and t == 0:
                    # startup: interleave the first weight tile's halves with
                    # x^T pieces so the first matmuls start ASAP.  Every xt
                    # piece MUST be emitted before t0's matmuls (the dep
                    # tracker only sees writes traced before a read).
                    nc.sync.dma_start(w[:, 0], dram.ap()[t][:, 0])
                    nc.sync.dma_start(xt[:, 0:2, :], xt_d.ap()[:, 0:2, :])
                    nc.sync.dma_start(w[:, 1], dram.ap()[t][:, 1])
                    nc.sync.dma_start(xt[:, 2:4, :], xt_d.ap()[:, 2:4, :])
                    nc.sync.dma_start(xt[:, 4:6, :], xt_d.ap()[:, 4:6, :])
                    nc.sync.dma_start(xt[:, 6:8, :], xt_d.ap()[:, 6:8, :])
                    for i in range(2, 4):
                        nc.sync.dma_start(xt[:, i * q:(i + 1) * q, :],
                                          xt_d.ap()[:, i * q:(i + 1) * q, :])
                else:
                    nc.sync.dma_start(w[:], dram.ap()[t])
                wgu_lists[p].append(w)

        def mk_wd(p, gs):
            dram = wd_drams[p]
            for g in gs:
                w = sb.tile([128, 2, IT, 128], bf16, tag="wd",
                            bufs=WD_BUFS, name="wd")
                nc.sync.dma_start(w[:], dram.ap()[g])
                wd_lists[p].append(w)

        # ---- queue B (gpsimd, SWDGE): gathered-x / cw / shared-gemm2
        # prefetches, led by a "gate" read of an early shared activation tile
        # (emitted after its producer, so the data dep holds the in-order
        # queue back) - they never race the startup or the weight streams.
        def b_part():
            # gate on the FIRST shared activation tile (ready ~28us); shared
            # t-tiles are 13.7us each, so gating on at[:,1] would hold wsd
            # past the shared-gemm2 start
            nc.gpsimd.dma_start(dum_d.ap()[:], ats[0][0:1, 0, 0:8])
            nc.gpsimd.dma_start(wsd[:], wsd_d.ap()[:])
            nc.gpsimd.dma_start(xgs[0][:], xg_d[0].ap()[:])
            nc.gpsimd.dma_start(cws[0][:], cw_d[0].ap()[:])
            nc.gpsimd.dma_start(xgs[1][:], xg_d[1].ap()[:])
            nc.gpsimd.dma_start(cws[1][:], cw_d[1].ap()[:])
            nc.gpsimd.dma_start(xgs[2][:], xg_d[2].ap()[:])
            nc.gpsimd.dma_start(cws[2][:], cw_d[2].ap()[:])

        def ffn(p, xg, cwB, out_dram, C, it_cnt, chunks,
                mixed_copy=False, og_sz=OG, post_t=None, post_og=None,
                split_last_out=False):
            at = ats[p]
            # GEMM1 + silu*up*cw -> at (activations^T, [i, tokens], bf16)
            for t in range(it_cnt):
                if t >= len(wgu_lists[p]):
                    mk_wgu(p, [t])
                wgu = wgu_lists[p][t]
                for off, n in chunks:
                    psg = ps.tile([128, n], f32, tag="psg", bufs=2, name="psg")
                    psu = ps.tile([128, n], f32, tag="psu", bufs=3, name="psu")
                    if p == 0 and t == 0:
                        # k-pair-interleaved chains: overlap with the
                        # arrival of the x^T pieces
                        for kq in range(8):
                            for j, pp in ((0, psg), (1, psu)):
                                for k in range(kq * 2, kq * 2 + 2):
                                    nc.tensor.matmul(
                                        pp[:], wgu[:, j, k, :],
                                        xg[:, k, off:off + n],
                                        start=(k == 0), stop=(k == KT - 1))
                    else:
                        for j, pp in ((0, psg), (1, psu)):
                            for k in range(KT):
                                nc.tensor.matmul(pp[:], wgu[:, j, k, :],
                                                 xg[:, k, off:off + n],
                                                 start=(k == 0),
                                                 stop=(k == KT - 1))
                    tmp = sb.tile([128, n], f32, tag="tmp", bufs=2, name="tmp")
                    nc.scalar.activation(tmp[:], psg[:], ACT_SILU)
                    if cwB is not None:
                        tmp2 = sb.tile([128, n], f32, tag="tmp2", bufs=2, name="tmp2")
                        nc.vector.tensor_mul(tmp2[:], tmp[:], psu[:])
                        nc.vector.tensor_mul(at[:, t, off:off + n], tmp2[:],
                                             cwB[:, off:off + n])
                    else:
                        nc.vector.tensor_mul(at[:, t, off:off + n], tmp[:], psu[:])
                if post_t is not None and t in post_t:
                    post_t[t]()
            # GEMM2 (transposed): y^T[d-tile] = sum_k wdT[dt,k].T @ at[k]
            wds = wd_lists[p]
            ci = 0
            for og in range(DT // og_sz):
                ysb = sb.tile([128, og_sz, C], bf16, tag="ysb", bufs=2, name="ysb")
                last = split_last_out and og == DT // og_sz - 1
                for e in range(og_sz):
                    dt = og * og_sz + e
                    wd = wds[dt // 2][:, dt % 2] if wds is not None \
                        else wsd[:, dt]
                    for off, n in chunks:
                        psy = ps.tile([128, n], f32, tag="psy", bufs=3, name="psy")
                        for k in range(it_cnt):
                            nc.tensor.matmul(psy[:], wd[:, k, :],
                                             at[:, k, off:off + n],
                                             start=(k == 0), stop=(k == it_cnt - 1))
                        if mixed_copy and ci % 2:
                            nc.scalar.activation(ysb[:, e, off:off + n], psy[:],
                                                 ACT_COPY)
                        else:
                            nc.vector.tensor_copy(ysb[:, e, off:off + n], psy[:])
                        ci += 1
                    if last and e == og_sz // 2 - 1:
                        # drain the first half early to shorten the kernel tail
                        nc.scalar.dma_start(out_dram.ap()[og][:, :og_sz // 2],
                                            ysb[:, :og_sz // 2])
                if last:
                    nc.scalar.dma_start(out_dram.ap()[og][:, og_sz // 2:],
                                        ysb[:, og_sz // 2:])
                else:
                    nc.scalar.dma_start(out_dram.ap()[og], ysb[:])
                if post_og is not None and og in post_og:
                    post_og[og]()

        ffn(0, xt, None, ys_d, T, SIT, [(0, 512), (512, 512)],
            mixed_copy=True, og_sz=OGS,
            post_t={1: b_part,
                    2: lambda: (mk_wgu(1, range(6)), mk_wd(1, range(6)))})
        ffn(1, xgs[0], cws[0], yr_d[0], caps[0], IT, [(0, caps[0])],
            post_og={0: lambda: (mk_wd(1, range(6, 8)),
                                 mk_wgu(2, range(3)), mk_wd(2, range(2)))})
        ffn(2, xgs[1], cws[1], yr_d[1], caps[1], IT, [(0, caps[1])],
            post_t={7: lambda: mk_wd(2, range(2, 4)),
                    8: lambda: mk_wd(2, range(4, 6))},
            post_og={0: lambda: (mk_wd(2, range(6, 8)),
                                 mk_wgu(3, range(3)), mk_wd(3, range(2)))})
        ffn(3, xgs[2], cws[2], yr_d[2], caps[2], IT, [(0, caps[2])],
            post_t={7: lambda: mk_wd(3, range(2, 4)),
                    8: lambda: mk_wd(3, range(4, 6))},
            post_og={0: lambda: mk_wd(3, range(6, 8))},
            split_last_out=True)

    nc.compile()
    return nc


# ----------------------------------------------------------------- kernel
def kernel(x, gate_w, bias, w_gate_up, w_down, shared_w_gate_up,
           shared_w_down, _trace=False):
    x = np.ascontiguousarray(x, dtype=np.float32)
    topk_idx, w = _route(x, gate_w, bias)
    cw_full = w.astype(np.float32) * np.float32(ROUTED_SCALE)

    # expert -> token list + weight list
    toks, wts, counts = [], [], np.zeros(E, dtype=np.int64)
    for e in range(E):
        tsel, ksel = np.where(topk_idx == e)
        toks.append(tsel)
        wts.append(cw_full[tsel, ksel])
        counts[e] = len(tsel)

    caps, uses = _solve_slots(counts)
    S = len(caps)
    # build per-class piece lists: (expert, token_idx_array, weight_array)
    class_pieces = [[] for _ in range(S)]
    for e in range(E):
        pos = 0
        # fill this expert's pieces largest-class first
        for s in range(S):
            for _ in range(uses[e][s]):
                n = min(caps[s], counts[e] - pos)
                class_pieces[s].append((e, toks[e][pos:pos + n], wts[e][pos:pos + n]))
                pos += n
    for s in range(S):
        while len(class_pieces[s]) < NCORES:   # dummy empty pieces
            class_pieces[s].append((0, toks[0][:0], wts[0][:0]))

    # pre-pack each expert's weights once (pieces share the arrays)
    wgu_pack = {}
    wd_pack = {}
    for s in range(S):
        for e, _, _ in class_pieces[s]:
            if e not in wgu_pack:
                wgu_pack[e] = _pack_wgu(w_gate_up[e], IT)
                wd_pack[e] = _pack_wdT(np.ascontiguousarray(w_down[e]), IT, 2)
    xt_arr = _pack_xT(x, T)

    in_maps = []
    for c in range(NCORES):
        m = {}
        for s in range(S):
            e, ptoks, pwts = class_pieces[s][c]
            m[f"xg{s}"] = _pack_xT(x[ptoks], caps[s])
            cwb = np.zeros((128, caps[s]), dtype=np.float32)
            cwb[:, :len(pwts)] = pwts[None, :]
            m[f"cw{s}"] = cwb
            m[f"wgu{s}"] = wgu_pack[e]
            m[f"wd{s}"] = wd_pack[e]
        # shared expert slice (rows [352c, 352c+352), zero-padded to 384)
        gsl = np.zeros((2 * SIP, D), dtype=np.float32)
        gsl[:SSLICE] = shared_w_gate_up[SSLICE * c: SSLICE * (c + 1)]
        gsl[SIP:SIP + SSLICE] = shared_w_gate_up[SI + SSLICE * c: SI + SSLICE * (c + 1)]
        m["wsgu"] = _pack_wgu(gsl, SIT)
        sds = np.zeros((D, SIP), dtype=np.float32)
        sds[:, :SSLICE] = shared_w_down[:, SSLICE * c: SSLICE * (c + 1)]
        # wsd as one partition-major array [128, DT, SIT, 128]
        m["wsd"] = np.ascontiguousarray(
            _pack_wdT(sds, SIT, 1).reshape(DT, 128, SIT, 128)
            .transpose(1, 0, 2, 3))
        m["xt"] = xt_arr
        in_maps.append(m)

    nc = _build(caps)
    kw = {}
    if _trace:
        kw = dict(trace=True, trace_cores=list(range(NCORES)))
    res = run_bass_kernel_spmd(nc, in_maps, core_ids=list(range(NCORES)), **kw)

    y = np.zeros((T, D), dtype=np.float32)
    for c in range(NCORES):
        y += _unpack_y(res.results[c]["ys"], T).T
    for c in range(NCORES):
        for s in range(S):
            e, ptoks, _ = class_pieces[s][c]
            n = len(ptoks)
            if n:
                y[ptoks] += _unpack_y(res.results[c][f"yr{s}"], caps[s])[:, :n].T
    if _trace:
        return y, res
    return y


# revision 33
# speedup vs baseline: 1.0162x; 1.0162x over previous
"""DeepseekV2 MoE layer on 8 Trainium2 NeuronCores (Bass/Tile, SPMD).

Strategy (expert-parallel, all-bf16 matmuls, fp32 PSUM accumulate):
 - Host computes the MoE gate routing in numpy (matches the jax reference:
   top-k margins are ~1e-4, far above ulp noise).
 - 16 experts -> 8 cores x 3 slots.  Slot capacities (C0>=C1>=C2) chosen
   by a small DP so expert token loads pack into the 24 slots with
   minimal total capacity; overfull experts are SPLIT across slots on
   different cores.  SPMD cores run an identical instruction stream, so
   per-core cost is C0+C1+C2 regardless of data.
 - Phase order: shared expert FIRST (its weight demand is only ~77 GB/s,
   so it absorbs the slow cold-start DMA window and gives the weight
   rings 60us of runway), then routed slots in descending cap; the small
   slot (which would need 408+ GB/s live) runs last, fully prefetched.
 - DMA scheduling: queue A (sync, HWDGE) carries every weight stream +
   x^T as ONE in-order deadline-sorted program of large partition-major
   transfers (>=4KB/partition line; small-descriptor DMAs run at ~1/3
   bandwidth).  In-order execution means a byte is never spent on a
   later-deadline transfer while an earlier one waits, and tile-ring WAR
   deps throttle run-ahead to the SBUF budget; each phase's first
   ring-depth of tiles is emitted from hook points in earlier phases.
   Queue B (gpsimd, SWDGE) carries gathered-x/cw/wsd prefetches behind a
   dummy "gate" read of an early activation tile so they cannot race the
   startup stream.  Outputs go on the scalar queue (with the silus),
   batched 8 d-tiles per DMA.  This removes the DGE descriptor
   contention and cross-queue bandwidth races of earlier versions.
 - Per slot: GEMM1 (wgu tiles stationary, gathered x^T moving) -> SiLU
   on ScalarE -> *up *cw on VectorE -> GEMM2 *transposed* (w_down^T
   stationary, activations moving) producing y^T [D, cap].  The routed
   gate weight (incl. the 2.5 scale) is applied via a host-replicated
   [128, cap] row.
 - Shared expert: TP-sharded over its intermediate dim (352/core, padded
   to 3x128), same pipeline; PSUM->SBUF copies alternate Vector/Scalar
   (vector alone rate-matches the tensor engine there and stutters).
 - bf16 operands halve HBM traffic vs fp32; measured rel err ~4e-3.
 - Host scatter-adds per-piece outputs and sums shared partials.
"""

import itertools
import numpy as np
import ml_dtypes
from contextlib import ExitStack

import concourse.bacc as bacc
import concourse.tile as tile
import concourse.mybir as mybir
from concourse.bass_utils import run_bass_kernel_spmd

# problem dims (fixed by the graded problem)
T, D, I, E = 1024, 2048, 1408, 16
SI = 2 * I               # shared expert intermediate (2816)
TOP_K, N_GROUP, TOPK_GROUP = 6, 4, 2
ROUTED_SCALE = 2.5
NCORES = 8
KT = D // 128            # 16 contraction tiles (gemm1)
IT = I // 128            # 11 intermediate tiles (routed)
DT = D // 128            # 16 output d-tiles (gemm2)
SSLICE = SI // NCORES    # 352 shared-intermediate rows per core
SIP = 384                # padded to 3x128
SIT = SIP // 128         # 3

f32 = mybir.dt.float32
bf16 = mybir.dt.bfloat16
BF = ml_dtypes.bfloat16
ACT_SILU = mybir.ActivationFunctionType.Silu
ACT_COPY = mybir.ActivationFunctionType.Copy

WGU_BUFS = 6             # wgu ring depth (cross-phase weight prefetch)
WD_BUFS = 6              # wd ring depth (2 d-tiles per buf)
OG = 8                   # d-tiles per output staging group (routed)
OGS = 2                  # d-tiles per output staging group (shared)


# ---------------------------------------------------------------- routing
def _route(x, gate_w, bias):
    """Replicates the jax reference gate in numpy f32 (decision margins are
    >=1e-4 so ulp-level differences cannot flip the top-k).

    Returns topk_idx [T,6] int, weights [T,6] f32 (renormalized, unscaled).
    """
    logits = (x @ gate_w.T).astype(np.float32)
    scores = (1.0 / (1.0 + np.exp(-logits))).astype(np.float32)
    s_choice = scores + bias.astype(np.float32)
    grp = s_choice.reshape(T, N_GROUP, E // N_GROUP)
    group_scores = np.sort(grp, axis=2)[:, :, -2:].sum(2, dtype=np.float32)
    grp_idx = np.argsort(-group_scores, axis=1, kind="stable")[:, :TOPK_GROUP]
    gmask = np.zeros((T, N_GROUP), dtype=bool)
    gmask[np.arange(T)[:, None], grp_idx] = True
    emask = np.repeat(gmask, E // N_GROUP, axis=1)
    masked = np.where(emask, s_choice, -np.inf)
    topk_idx = np.argsort(-masked, axis=1, kind="stable")[:, :TOP_K]
    w = np.take_along_axis(scores, topk_idx, axis=1)
    w = (w / w.sum(axis=1, keepdims=True)).astype(np.float32)
    return topk_idx, w


# --------------------------------------------------------- slot assignment
def _feasible(caps, counts, max_pieces=3):
    """Can each expert be covered by <=max_pieces slots (8 per capacity
    class) with total capacity >= its token count?  Returns per-expert
    class-use tuples or None."""
    nclass = len(caps)
    opts_per_expert = []
    for cnt in counts:
        opts = []
        for a in itertools.product(range(max_pieces + 1), repeat=nclass):
            if sum(a) == 0 or sum(a) > max_pieces:
                continue
            tot = sum(ai * c for ai, c in zip(a, caps))
            if tot >= cnt:
                dom = any(a[j] > 0 and tot - caps[j] >= cnt for j in range(nclass))
                if not dom:
                    opts.append(a)
        if not opts:
            return None
        opts_per_expert.append(opts)
    states = {(0,) * nclass: []}
    for opts in opts_per_expert:
        new = {}
        for st, hist in states.items():
            for a in opts:
                nst = tuple(s + ai for s, ai in zip(st, a))
                if all(n <= NCORES for n in nst) and nst not in new:
                    new[nst] = hist + [a]
        if not new:
            return None
        states = new
    return next(iter(states.values()))


def _solve_slots(counts):
    """Pick 3 slot capacities (multiples of 8, each <=512) minimizing
    C0+C1+C2 s.t. the expert loads pack into 8 slots per class.
    Returns (caps, per-expert class-use tuples)."""
    total = int(counts.sum())
    lo = -(-total // NCORES)
    lo = -(-lo // 8) * 8
    for s in range(lo, 1537, 8):
        for C0 in range(-(-s // 3) // 8 * 8, min(512, s - 16) + 1, 8):
            for C1 in range(8, min(C0, s - C0 - 8) + 1, 8):
                C2 = s - C0 - C1
                if C2 < 8 or C2 > C1:
                    continue
                r = _feasible((C0, C1, C2), counts)
                if r is not None:
                    return [C0, C1, C2], r
    raise RuntimeError("no feasible slot packing")


# ------------------------------------------------------------ host packing
def _pack_wgu(w2, it_cnt):
    """w2: [2*ic, D] rows (gate block then up block, ic=128*it_cnt rows)
    -> [it_cnt, 128, 2, KT, 128] bf16: tile (t, j) is wgu^T laid out so
    [p, c] = w2[j_block + t*128 + c, k*128 + p] (lhsT k-slices)."""
    ic = 128 * it_cnt
    g = w2[:ic].reshape(it_cnt, 128, D)
    u = w2[ic:].reshape(it_cnt, 128, D)
    arr = np.stack([g, u], axis=1)               # (t, j, c, D)
    arr = arr.reshape(it_cnt, 2, 128, KT, 128)   # (t, j, c, k, p)
    return np.ascontiguousarray(arr.transpose(0, 4, 1, 3, 2), dtype=BF)


def _pack_wdT(wd, it_cnt, grp):
    """wd: [D, 128*it_cnt] (w_down, cols may be zero padded)
    -> [DT//grp, 128, grp, it_cnt, 128] bf16: tile (g)[:, e, k, :] is the
    lhsT for d-tile dt=g*grp+e: [i, d] = wd[dt*128 + d, k*128 + i]."""
    a = wd.reshape(DT, 128, it_cnt, 128)         # (dt, d, k, i)
    a = a.transpose(0, 3, 2, 1)                  # (dt, i, k, d)
    a = a.reshape(DT // grp, grp, 128, it_cnt, 128)
    return np.ascontiguousarray(a.transpose(0, 2, 1, 3, 4), dtype=BF)


def _pack_xT(xs, cap):
    """xs: [n, D] token rows -> [128, KT, cap] bf16 (x^T k-tiles, padded)."""
    out = np.zeros((128, KT, cap), dtype=BF)
    n = xs.shape[0]
    if n:
        out[:, :, :n] = xs.T.reshape(KT, 128, n).transpose(1, 0, 2).astype(BF)
    return out


def _unpack_y(arr, cap):
    """[DT//og, 128, og, cap] bf16 device output -> [D, cap] f32."""
    return arr.transpose(0, 2, 1, 3).reshape(D, cap).astype(np.float32)


# ------------------------------------------------------------ device build
def _build(caps):
    nc = bacc.Bacc("TRN2", target_bir_lowering=False, debug=False,
                   num_devices=NCORES)
    S = len(caps)
    xg_d = [nc.dram_tensor(f"xg{s}", [128, KT, caps[s]], bf16, kind="ExternalInput") for s in range(S)]
    cw_d = [nc.dram_tensor(f"cw{s}", [128, caps[s]], f32, kind="ExternalInput") for s in range(S)]
    wgu_d = [nc.dram_tensor(f"wgu{s}", [IT, 128, 2, KT, 128], bf16, kind="ExternalInput") for s in range(S)]
    wd_d = [nc.dram_tensor(f"wd{s}", [DT // 2, 128, 2, IT, 128], bf16, kind="ExternalInput") for s in range(S)]
    yr_d = [nc.dram_tensor(f"yr{s}", [DT // OG, 128, OG, caps[s]], bf16, kind="ExternalOutput") for s in range(S)]
    xt_d = nc.dram_tensor("xt", [2, 128, KT, 512], bf16, kind="ExternalInput")
    wsgu_d = nc.dram_tensor("wsgu", [SIT, 128, 2, KT, 128], bf16, kind="ExternalInput")
    wsd_d = nc.dram_tensor("wsd", [128, DT, SIT, 128], bf16, kind="ExternalInput")
    ys_d = nc.dram_tensor("ys", [DT // OGS, 128, OGS, T], bf16, kind="ExternalOutput")
    dum_d = nc.dram_tensor("dum", [1, 8], bf16, kind="Internal")

    with tile.TileContext(nc) as tc, ExitStack() as ctx:
        sb = ctx.enter_context(tc.tile_pool(name="sb", bufs=1))
        ps = ctx.enter_context(tc.tile_pool(name="ps", bufs=1, space="PSUM"))

        q = KT // 4
        xgs = [sb.tile([128, KT, caps[s]], bf16, tag=f"xg{s}", bufs=1,
                       name=f"xga{s}") for s in range(S)]
        cws = [sb.tile([128, caps[s]], f32, tag=f"cw{s}", bufs=1,
                       name=f"cwa{s}") for s in range(S)]
        xt = sb.tile([128, 2, KT, 512], bf16, tag="xt", bufs=1, name="xt")
        wsd = sb.tile([128, DT, SIT, 128], bf16, tag="wsd", bufs=1, name="wsd")
        # phase order: shared, slot0 (344), slot1 (280), slot2 (192)
        ats = [sb.tile([128, SIT, T], bf16, tag="at", bufs=2, name="atsh"),
               sb.tile([128, IT, caps[0]], bf16, tag="at", bufs=2, name="at0"),
               sb.tile([128, IT, caps[1]], bf16, tag="at", bufs=2, name="at1"),
               sb.tile([128, IT, caps[2]], bf16, tag="at", bufs=2, name="at2")]

        # ---- queue A (sync, HWDGE): all weight streams + x^T, one in-order
        # deadline-sorted program.  In-order execution means a byte is never
        # spent on a later-deadline transfer while an earlier-deadline one
        # waits; ring WAR deps throttle run-ahead to the SBUF budget.  Each
        # phase's first ring-depth of tiles is emitted from hook points in
        # earlier phases (prefetch), the rest from its own loops.  All
        # transfers are >=4KB per partition line - small-descriptor DMAs run
        # at ~1/3 bandwidth and strangle the in-order queue.
        wgu_lists = [[], [], [], []]
        wd_lists = [None, [], [], []]
        wgu_drams = [wsgu_d, wgu_d[0], wgu_d[1], wgu_d[2]]
        wd_drams = [None, wd_d[0], wd_d[1], wd_d[2]]

        def mk_wgu(p, ts):
            dram = wgu_drams[p]
            for t in ts:
                w = sb.tile([128, 2, KT, 128], bf16, tag="wgu",
                            bufs=WGU_BUFS, name="wgu")
                nc.sync.dma_start(w[:], dram.ap()[t])
                wgu_lists[p].append(w)

        def emit_shared_head():
            # startup: interleave the first weight tile's halves with the
            # chunk-0 x^T quarters so the first matmuls start on a ~1MB
            # critical head; chunk-1 x^T follows with a 20us-later deadline.
            # Every piece MUST be emitted before the matmuls that read it
            # (the dep tracker only sees writes traced before a read).
            w0 = sb.tile([128, 2, KT, 128], bf16, tag="wgu",
                         bufs=WGU_BUFS, name="wgu")
            nc.sync.dma_start(w0[:, 0], wgu_drams[0].ap()[0][:, 0])
            nc.sync.dma_start(xt[:, 0, 0:4, :], xt_d.ap()[0][:, 0:4, :])
            nc.sync.dma_start(w0[:, 1], wgu_drams[0].ap()[0][:, 1])
            for i in range(1, 4):
                nc.sync.dma_start(xt[:, 0, i * 4:(i + 1) * 4, :],
                                  xt_d.ap()[0][:, i * 4:(i + 1) * 4, :])
            wgu_lists[0].append(w0)
            mk_wgu(0, range(1, SIT))
            for i in range(4):
                nc.sync.dma_start(xt[:, 1, i * 4:(i + 1) * 4, :],
                                  xt_d.ap()[1][:, i * 4:(i + 1) * 4, :])

        def mk_wd(p, gs):
            dram = wd_drams[p]
            for g in gs:
                w = sb.tile([128, 2, IT, 128], bf16, tag="wd",
                            bufs=WD_BUFS, name="wd")
                nc.sync.dma_start(w[:], dram.ap()[g])
                wd_lists[p].append(w)

        # ---- queue B (gpsimd, SWDGE): gathered-x / cw / shared-gemm2
        # prefetches, led by a "gate" read of an early shared activation tile
        # (emitted after its producer, so the data dep holds the in-order
        # queue back) - they never race the startup or the weight streams.
        def b_part():
            # gate on the FIRST shared activation tile (ready ~28us); shared
            # t-tiles are 13.7us each, so gating on at[:,1] would hold wsd
            # past the shared-gemm2 start
            nc.gpsimd.dma_start(dum_d.ap()[:], ats[0][0:1, 0, 0:8])
            nc.gpsimd.dma_start(wsd[:], wsd_d.ap()[:])
            nc.gpsimd.dma_start(xgs[0][:], xg_d[0].ap()[:])
            nc.gpsimd.dma_start(cws[0][:], cw_d[0].ap()[:])
            nc.gpsimd.dma_start(xgs[1][:], xg_d[1].ap()[:])
            nc.gpsimd.dma_start(cws[1][:], cw_d[1].ap()[:])
            nc.gpsimd.dma_start(xgs[2][:], xg_d[2].ap()[:])
            nc.gpsimd.dma_start(cws[2][:], cw_d[2].ap()[:])

        def ffn(p, xg, cwB, out_dram, C, it_cnt, chunks,
                mixed_copy=False, og_sz=OG, post_t=None, post_og=None,
                split_last_out=False):
            at = ats[p]

            def act(t, off, n, psg, psu):
                tmp = sb.tile([128, n], f32, tag="tmp", bufs=2, name="tmp")
                nc.scalar.activation(tmp[:], psg[:], ACT_SILU)
                if cwB is not None:
                    tmp2 = sb.tile([128, n], f32, tag="tmp2", bufs=2, name="tmp2")
                    nc.vector.tensor_mul(tmp2[:], tmp[:], psu[:])
                    nc.vector.tensor_mul(at[:, t, off:off + n], tmp2[:],
                                         cwB[:, off:off + n])
                else:
                    nc.vector.tensor_mul(at[:, t, off:off + n], tmp[:], psu[:])

            # GEMM1 + silu*up*cw -> at (activations^T, [i, tokens], bf16)
            if p == 0:
                # chunk-outer: all three weight tiles sweep chunk-0 columns
                # first (small cold-start head, chunk-1 x^T deadline +20us);
                # weight tiles stay resident so nothing is re-loaded.
                emit_shared_head()
                for ci, (off, n) in enumerate(chunks):
                    for t in range(it_cnt):
                        wgu = wgu_lists[0][t]
                        psg = ps.tile([128, n], f32, tag="psg", bufs=2, name="psg")
                        psu = ps.tile([128, n], f32, tag="psu", bufs=3, name="psu")
                        if ci == 0 and t == 0:
                            # k-pair-interleaved chains: overlap the arrival
                            # of the chunk-0 x^T quarters
                            for kq in range(8):
                                for j, pp in ((0, psg), (1, psu)):
                                    for k in range(kq * 2, kq * 2 + 2):
                                        nc.tensor.matmul(
                                            pp[:], wgu[:, j, k, :],
                                            xg[:, ci, k, 0:n],
                                            start=(k == 0), stop=(k == KT - 1))
                        else:
                            for j, pp in ((0, psg), (1, psu)):
                                for k in range(KT):
                                    nc.tensor.matmul(pp[:], wgu[:, j, k, :],
                                                     xg[:, ci, k, 0:n],
                                                     start=(k == 0),
                                                     stop=(k == KT - 1))
                        act(t, off, n, psg, psu)
                        if ci == 0 and t == 0:
                            b_part()
                # next-phase weight prefetch: emitted only now because the
                # recycled ring slots' readers span BOTH chunk passes
                mk_wgu(1, range(6))
                mk_wd(1, range(6))
            else:
                for t in range(it_cnt):
                    if t >= len(wgu_lists[p]):
                        mk_wgu(p, [t])
                    wgu = wgu_lists[p][t]
                    for off, n in chunks:
                        psg = ps.tile([128, n], f32, tag="psg", bufs=2, name="psg")
                        psu = ps.tile([128, n], f32, tag="psu", bufs=3, name="psu")
                        for j, pp in ((0, psg), (1, psu)):
                            for k in range(KT):
                                nc.tensor.matmul(pp[:], wgu[:, j, k, :],
                                                 xg[:, k, off:off + n],
                                                 start=(k == 0),
                                                 stop=(k == KT - 1))
                        act(t, off, n, psg, psu)
                    if post_t is not None and t in post_t:
                        post_t[t]()
            # GEMM2 (transposed): y^T[d-tile] = sum_k wdT[dt,k].T @ at[k]
            wds = wd_lists[p]
            ci = 0
            for og in range(DT // og_sz):
                ysb = sb.tile([128, og_sz, C], bf16, tag="ysb", bufs=2, name="ysb")
                last = split_last_out and og == DT // og_sz - 1
                for e in range(og_sz):
                    dt = og * og_sz + e
                    wd = wds[dt // 2][:, dt % 2] if wds is not None \
                        else wsd[:, dt]
                    for off, n in chunks:
                        psy = ps.tile([128, n], f32, tag="psy", bufs=3, name="psy")
                        for k in range(it_cnt):
                            nc.tensor.matmul(psy[:], wd[:, k, :],
                                             at[:, k, off:off + n],
                                             start=(k == 0), stop=(k == it_cnt - 1))
                        if mixed_copy and ci % 2:
                            nc.scalar.activation(ysb[:, e, off:off + n], psy[:],
                                                 ACT_COPY)
                        else:
                            nc.vector.tensor_copy(ysb[:, e, off:off + n], psy[:])
                        ci += 1
                    if last and e == og_sz // 2 - 1:
                        # drain the first half early to shorten the kernel tail
                        nc.scalar.dma_start(out_dram.ap()[og][:, :og_sz // 2],
                                            ysb[:, :og_sz // 2])
                if last:
                    nc.scalar.dma_start(out_dram.ap()[og][:, og_sz // 2:],
                                        ysb[:, og_sz // 2:])
                else:
                    nc.scalar.dma_start(out_dram.ap()[og], ysb[:])
                if post_og is not None and og in post_og:
                    post_og[og]()

        ffn(0, xt, None, ys_d, T, SIT, [(0, 512), (512, 512)],
            mixed_copy=True, og_sz=OGS)
        ffn(1, xgs[0], cws[0], yr_d[0], caps[0], IT, [(0, caps[0])],
            post_og={0: lambda: (mk_wd(1, range(6, 8)),
                                 mk_wgu(2, range(3)), mk_wd(2, range(2)))})
        ffn(2, xgs[1], cws[1], yr_d[1], caps[1], IT, [(0, caps[1])],
            post_t={7: lambda: mk_wd(2, range(2, 4)),
                    8: lambda: mk_wd(2, range(4, 6))},
            post_og={0: lambda: (mk_wd(2, range(6, 8)),
                                 mk_wgu(3, range(3)), mk_wd(3, range(2)))})
        ffn(3, xgs[2], cws[2], yr_d[2], caps[2], IT, [(0, caps[2])],
            post_t={7: lambda: mk_wd(3, range(2, 4)),
                    8: lambda: mk_wd(3, range(4, 6))},
            post_og={0: lambda: mk_wd(3, range(6, 8))},
            split_last_out=True)

    nc.compile()
    return nc


# ----------------------------------------------------------------- kernel
def kernel(x, gate_w, bias, w_gate_up, w_down, shared_w_gate_up,
           shared_w_down, _trace=False):
    x = np.ascontiguousarray(x, dtype=np.float32)
    topk_idx, w = _route(x, gate_w, bias)
    cw_full = w.astype(np.float32) * np.float32(ROUTED_SCALE)

    # expert -> token list + weight list
    toks, wts, counts = [], [], np.zeros(E, dtype=np.int64)
    for e in range(E):
        tsel, ksel = np.where(topk_idx == e)
        toks.append(tsel)
        wts.append(cw_full[tsel, ksel])
        counts[e] = len(tsel)

    caps, uses = _solve_slots(counts)
    S = len(caps)
    # build per-class piece lists: (expert, token_idx_array, weight_array)
    class_pieces = [[] for _ in range(S)]
    for e in range(E):
        pos = 0
        # fill this expert's pieces largest-class first
        for s in range(S):
            for _ in range(uses[e][s]):
                n = min(caps[s], counts[e] - pos)
                class_pieces[s].append((e, toks[e][pos:pos + n], wts[e][pos:pos + n]))
                pos += n
    for s in range(S):
        while len(class_pieces[s]) < NCORES:   # dummy empty pieces
            class_pieces[s].append((0, toks[0][:0], wts[0][:0]))

    # pre-pack each expert's weights once (pieces share the arrays)
    wgu_pack = {}
    wd_pack = {}
    for s in range(S):
        for e, _, _ in class_pieces[s]:
            if e not in wgu_pack:
                wgu_pack[e] = _pack_wgu(w_gate_up[e], IT)
                wd_pack[e] = _pack_wdT(np.ascontiguousarray(w_down[e]), IT, 2)
    xt_arr = _pack_xT(x, T)

    in_maps = []
    for c in range(NCORES):
        m = {}
        for s in range(S):
            e, ptoks, pwts = class_pieces[s][c]
            m[f"xg{s}"] = _pack_xT(x[ptoks], caps[s])
            cwb = np.zeros((128, caps[s]), dtype=np.float32)
            cwb[:, :len(pwts)] = pwts[None, :]
            m[f"cw{s}"] = cwb
            m[f"wgu{s}"] = wgu_pack[e]
            m[f"wd{s}"] = wd_pack[e]
        # shared expert slice (rows [352c, 352c+352), zero-padded to 384)
        gsl = np.zeros((2 * SIP, D), dtype=np.float32)
        gsl[:SSLICE] = shared_w_gate_up[SSLICE * c: SSLICE * (c + 1)]
        gsl[SIP:SIP + SSLICE] = shared_w_gate_up[SI + SSLICE * c: SI + SSLICE * (c + 1)]
        m["wsgu"] = _pack_wgu(gsl, SIT)
        sds = np.zeros((D, SIP), dtype=np.float32)
        sds[:, :SSLICE] = shared_w_down[:, SSLICE * c: SSLICE * (c + 1)]
        # wsd as one partition-major array [128, DT, SIT, 128]
        m["wsd"] = np.ascontiguousarray(
            _pack_wdT(sds, SIT, 1).reshape(DT, 128, SIT, 128)
            .transpose(1, 0, 2, 3))
        m["xt"] = np.stack([xt_arr[:, :, :512], xt_arr[:, :, 512:]], axis=0)
        in_maps.append(m)

    nc = _build(caps)
    kw = {}
    if _trace:
        kw = dict(trace=True, trace_cores=list(range(NCORES)))
    res = run_bass_kernel_spmd(nc, in_maps, core_ids=list(range(NCORES)), **kw)

    y = np.zeros((T, D), dtype=np.float32)
    for c in range(NCORES):
        y += _unpack_y(res.results[c]["ys"], T).T
    for c in range(NCORES):
        for s in range(S):
            e, ptoks, _ = class_pieces[s][c]
            n = len(ptoks)
            if n:
                y[ptoks] += _unpack_y(res.results[c][f"yr{s}"], caps[s])[:, :n].T
    if _trace:
        return y, res
    return y


# revision 34
# speedup vs baseline: 1.0247x; 1.0084x over previous
"""DeepseekV2 MoE layer on 8 Trainium2 NeuronCores (Bass/Tile, SPMD).

Strategy (expert-parallel, all-bf16 matmuls, fp32 PSUM accumulate):
 - Host computes the MoE gate routing in numpy (matches the jax reference:
   top-k margins are ~1e-4, far above ulp noise).
 - 16 experts -> 8 cores x 3 slots.  Slot capacities (C0>=C1>=C2) chosen
   by a small DP so expert token loads pack into the 24 slots with
   minimal total capacity; overfull experts are SPLIT across slots on
   different cores.  SPMD cores run an identical instruction stream, so
   per-core cost is C0+C1+C2 regardless of data.
 - Phase order: shared expert FIRST (its weight demand is only ~77 GB/s,
   so it absorbs the slow cold-start DMA window and gives the weight
   rings 60us of runway), then routed slots in descending cap; the small
   slot (which would need 408+ GB/s live) runs last, fully prefetched.
 - DMA scheduling: queue A (sync, HWDGE) carries every weight stream +
   x^T as ONE in-order deadline-sorted program of large partition-major
   transfers (>=4KB/partition line; small-descriptor DMAs run at ~1/3
   bandwidth).  In-order execution means a byte is never spent on a
   later-deadline transfer while an earlier one waits, and tile-ring WAR
   deps throttle run-ahead to the SBUF budget; each phase's first
   ring-depth of tiles is emitted from hook points in earlier phases.
   Queue B (gpsimd, SWDGE) carries gathered-x/cw/wsd prefetches behind a
   dummy "gate" read of an early activation tile so they cannot race the
   startup stream.  Outputs go on the scalar queue (with the silus),
   batched 8 d-tiles per DMA.  This removes the DGE descriptor
   contention and cross-queue bandwidth races of earlier versions.
 - Per slot: GEMM1 (wgu tiles stationary, gathered x^T moving) -> SiLU
   on ScalarE -> *up *cw on VectorE -> GEMM2 *transposed* (w_down^T
   stationary, activations moving) producing y^T [D, cap].  The routed
   gate weight (incl. the 2.5 scale) is applied via a host-replicated
   [128, cap] row.
 - Shared expert: TP-sharded over its intermediate dim (352/core, padded
   to 3x128), same pipeline; PSUM->SBUF copies alternate Vector/Scalar
   (vector alone rate-matches the tensor engine there and stutters).
 - bf16 operands halve HBM traffic vs fp32; measured rel err ~4e-3.
 - Host scatter-adds per-piece outputs and sums shared partials.
"""

import itertools
import numpy as np
import ml_dtypes
from contextlib import ExitStack

import concourse.bacc as bacc
import concourse.tile as tile
import concourse.mybir as mybir
from concourse.bass_utils import run_bass_kernel_spmd

# problem dims (fixed by the graded problem)
T, D, I, E = 1024, 2048, 1408, 16
SI = 2 * I               # shared expert intermediate (2816)
TOP_K, N_GROUP, TOPK_GROUP = 6, 4, 2
ROUTED_SCALE = 2.5
NCORES = 8
KT = D // 128            # 16 contraction tiles (gemm1)
IT = I // 128            # 11 intermediate tiles (routed)
DT = D // 128            # 16 output d-tiles (gemm2)
SSLICE = SI // NCORES    # 352 shared-intermediate rows per core
SIP = 384                # padded to 3x128
SIT = SIP // 128         # 3

f32 = mybir.dt.float32
bf16 = mybir.dt.bfloat16
BF = ml_dtypes.bfloat16
ACT_SILU = mybir.ActivationFunctionType.Silu
ACT_COPY = mybir.ActivationFunctionType.Copy

WGU_BUFS = 6             # wgu ring depth (cross-phase weight prefetch)
WD_BUFS = 6              # wd ring depth (2 d-tiles per buf)
OG = 8                   # d-tiles per output staging group (routed)
OGS = 2                  # d-tiles per output staging group (shared)


# ---------------------------------------------------------------- routing
def _route(x, gate_w, bias):
    """Replicates the jax reference gate in numpy f32 (decision margins are
    >=1e-4 so ulp-level differences cannot flip the top-k).

    Returns topk_idx [T,6] int, weights [T,6] f32 (renormalized, unscaled).
    """
    logits = (x @ gate_w.T).astype(np.float32)
    scores = (1.0 / (1.0 + np.exp(-logits))).astype(np.float32)
    s_choice = scores + bias.astype(np.float32)
    grp = s_choice.reshape(T, N_GROUP, E // N_GROUP)
    group_scores = np.sort(grp, axis=2)[:, :, -2:].sum(2, dtype=np.float32)
    grp_idx = np.argsort(-group_scores, axis=1, kind="stable")[:, :TOPK_GROUP]
    gmask = np.zeros((T, N_GROUP), dtype=bool)
    gmask[np.arange(T)[:, None], grp_idx] = True
    emask = np.repeat(gmask, E // N_GROUP, axis=1)
    masked = np.where(emask, s_choice, -np.inf)
    topk_idx = np.argsort(-masked, axis=1, kind="stable")[:, :TOP_K]
    w = np.take_along_axis(scores, topk_idx, axis=1)
    w = (w / w.sum(axis=1, keepdims=True)).astype(np.float32)
    return topk_idx, w


# --------------------------------------------------------- slot assignment
def _feasible(caps, counts, max_pieces=3):
    """Can each expert be covered by <=max_pieces slots (8 per capacity
    class) with total capacity >= its token count?  Returns per-expert
    class-use tuples or None."""
    nclass = len(caps)
    opts_per_expert = []
    for cnt in counts:
        opts = []
        for a in itertools.product(range(max_pieces + 1), repeat=nclass):
            if sum(a) == 0 or sum(a) > max_pieces:
                continue
            tot = sum(ai * c for ai, c in zip(a, caps))
            if tot >= cnt:
                dom = any(a[j] > 0 and tot - caps[j] >= cnt for j in range(nclass))
                if not dom:
                    opts.append(a)
        if not opts:
            return None
        opts_per_expert.append(opts)
    states = {(0,) * nclass: []}
    for opts in opts_per_expert:
        new = {}
        for st, hist in states.items():
            for a in opts:
                nst = tuple(s + ai for s, ai in zip(st, a))
                if all(n <= NCORES for n in nst) and nst not in new:
                    new[nst] = hist + [a]
        if not new:
            return None
        states = new
    return next(iter(states.values()))


def _solve_slots(counts):
    """Pick 3 slot capacities (multiples of 8, each <=512) minimizing
    C0+C1+C2 s.t. the expert loads pack into 8 slots per class.
    Returns (caps, per-expert class-use tuples)."""
    total = int(counts.sum())
    lo = -(-total // NCORES)
    lo = -(-lo // 8) * 8
    for s in range(lo, 1537, 8):
        for C0 in range(-(-s // 3) // 8 * 8, min(512, s - 16) + 1, 8):
            for C1 in range(8, min(C0, s - C0 - 8) + 1, 8):
                C2 = s - C0 - C1
                if C2 < 8 or C2 > C1:
                    continue
                r = _feasible((C0, C1, C2), counts)
                if r is not None:
                    return [C0, C1, C2], r
    raise RuntimeError("no feasible slot packing")


# ------------------------------------------------------------ host packing
def _pack_wgu(w2, it_cnt):
    """w2: [2*ic, D] rows (gate block then up block, ic=128*it_cnt rows)
    -> [it_cnt, 128, 2, KT, 128] bf16: tile (t, j) is wgu^T laid out so
    [p, c] = w2[j_block + t*128 + c, k*128 + p] (lhsT k-slices)."""
    ic = 128 * it_cnt
    g = w2[:ic].reshape(it_cnt, 128, D)
    u = w2[ic:].reshape(it_cnt, 128, D)
    arr = np.stack([g, u], axis=1)               # (t, j, c, D)
    arr = arr.reshape(it_cnt, 2, 128, KT, 128)   # (t, j, c, k, p)
    return np.ascontiguousarray(arr.transpose(0, 4, 1, 3, 2), dtype=BF)


def _pack_wdT(wd, it_cnt, grp):
    """wd: [D, 128*it_cnt] (w_down, cols may be zero padded)
    -> [DT//grp, 128, grp, it_cnt, 128] bf16: tile (g)[:, e, k, :] is the
    lhsT for d-tile dt=g*grp+e: [i, d] = wd[dt*128 + d, k*128 + i]."""
    a = wd.reshape(DT, 128, it_cnt, 128)         # (dt, d, k, i)
    a = a.transpose(0, 3, 2, 1)                  # (dt, i, k, d)
    a = a.reshape(DT // grp, grp, 128, it_cnt, 128)
    return np.ascontiguousarray(a.transpose(0, 2, 1, 3, 4), dtype=BF)


def _pack_xT(xs, cap):
    """xs: [n, D] token rows -> [128, KT, cap] bf16 (x^T k-tiles, padded)."""
    out = np.zeros((128, KT, cap), dtype=BF)
    n = xs.shape[0]
    if n:
        out[:, :, :n] = xs.T.reshape(KT, 128, n).transpose(1, 0, 2).astype(BF)
    return out


def _unpack_y(arr, cap):
    """[DT//og, 128, og, cap] bf16 device output -> [D, cap] f32."""
    return arr.transpose(0, 2, 1, 3).reshape(D, cap).astype(np.float32)


# ------------------------------------------------------------ device build
def _build(caps):
    nc = bacc.Bacc("TRN2", target_bir_lowering=False, debug=False,
                   num_devices=NCORES)
    S = len(caps)
    xg_d = [nc.dram_tensor(f"xg{s}", [128, KT, caps[s]], bf16, kind="ExternalInput") for s in range(S)]
    cw_d = [nc.dram_tensor(f"cw{s}", [128, caps[s]], f32, kind="ExternalInput") for s in range(S)]
    wgu_d = [nc.dram_tensor(f"wgu{s}", [IT, 128, 2, KT, 128], bf16, kind="ExternalInput") for s in range(S)]
    wd_d = [nc.dram_tensor(f"wd{s}", [DT // 2, 128, 2, IT, 128], bf16, kind="ExternalInput") for s in range(S)]
    yr_d = [nc.dram_tensor(f"yr{s}", [DT // OG, 128, OG, caps[s]], bf16, kind="ExternalOutput") for s in range(S)]
    xt_d = nc.dram_tensor("xt", [2, 128, KT, 512], bf16, kind="ExternalInput")
    wsgu_d = nc.dram_tensor("wsgu", [SIT, 128, 2, KT, 128], bf16, kind="ExternalInput")
    wsd_d = nc.dram_tensor("wsd", [128, DT, SIT, 128], bf16, kind="ExternalInput")
    ys_d = nc.dram_tensor("ys", [DT // OGS, 128, OGS, T], bf16, kind="ExternalOutput")
    dum_d = nc.dram_tensor("dum", [1, 8], bf16, kind="Internal")

    with tile.TileContext(nc) as tc, ExitStack() as ctx:
        sb = ctx.enter_context(tc.tile_pool(name="sb", bufs=1))
        ps = ctx.enter_context(tc.tile_pool(name="ps", bufs=1, space="PSUM"))

        q = KT // 4
        xgs = [sb.tile([128, KT, caps[s]], bf16, tag=f"xg{s}", bufs=1,
                       name=f"xga{s}") for s in range(S)]
        cws = [sb.tile([128, caps[s]], f32, tag=f"cw{s}", bufs=1,
                       name=f"cwa{s}") for s in range(S)]
        xt = sb.tile([128, 2, KT, 512], bf16, tag="xt", bufs=1, name="xt")
        wsd = sb.tile([128, DT, SIT, 128], bf16, tag="wsd", bufs=1, name="wsd")
        # phase order: shared, slot0 (344), slot1 (280), slot2 (192)
        ats = [sb.tile([128, SIT, T], bf16, tag="at", bufs=2, name="atsh"),
               sb.tile([128, IT, caps[0]], bf16, tag="at", bufs=2, name="at0"),
               sb.tile([128, IT, caps[1]], bf16, tag="at", bufs=2, name="at1"),
               sb.tile([128, IT, caps[2]], bf16, tag="at", bufs=2, name="at2")]

        # ---- queue A (sync, HWDGE): all weight streams + x^T, one in-order
        # deadline-sorted program.  In-order execution means a byte is never
        # spent on a later-deadline transfer while an earlier-deadline one
        # waits; ring WAR deps throttle run-ahead to the SBUF budget.  Each
        # phase's first ring-depth of tiles is emitted from hook points in
        # earlier phases (prefetch), the rest from its own loops.  All
        # transfers are >=4KB per partition line - small-descriptor DMAs run
        # at ~1/3 bandwidth and strangle the in-order queue.
        wgu_lists = [[], [], [], []]
        wd_lists = [None, [], [], []]
        wgu_drams = [wsgu_d, wgu_d[0], wgu_d[1], wgu_d[2]]
        wd_drams = [None, wd_d[0], wd_d[1], wd_d[2]]

        def mk_wgu(p, ts):
            dram = wgu_drams[p]
            for t in ts:
                w = sb.tile([128, 2, KT, 128], bf16, tag="wgu",
                            bufs=WGU_BUFS, name="wgu")
                nc.sync.dma_start(w[:], dram.ap()[t])
                wgu_lists[p].append(w)

        def emit_shared_head():
            # startup: interleave the first weight tile's halves with the
            # chunk-0 x^T quarters so the first matmuls start on a ~1MB
            # critical head; chunk-1 x^T follows with a 20us-later deadline.
            # Every piece MUST be emitted before the matmuls that read it
            # (the dep tracker only sees writes traced before a read).
            w0 = sb.tile([128, 2, KT, 128], bf16, tag="wgu",
                         bufs=WGU_BUFS, name="wgu")
            nc.sync.dma_start(w0[:, 0], wgu_drams[0].ap()[0][:, 0])
            nc.sync.dma_start(xt[:, 0, 0:4, :], xt_d.ap()[0][:, 0:4, :])
            nc.sync.dma_start(w0[:, 1], wgu_drams[0].ap()[0][:, 1])
            for i in range(1, 4):
                nc.sync.dma_start(xt[:, 0, i * 4:(i + 1) * 4, :],
                                  xt_d.ap()[0][:, i * 4:(i + 1) * 4, :])
            wgu_lists[0].append(w0)
            mk_wgu(0, range(1, SIT))
            for i in range(4):
                nc.sync.dma_start(xt[:, 1, i * 4:(i + 1) * 4, :],
                                  xt_d.ap()[1][:, i * 4:(i + 1) * 4, :])

        def mk_wd(p, gs):
            dram = wd_drams[p]
            for g in gs:
                w = sb.tile([128, 2, IT, 128], bf16, tag="wd",
                            bufs=WD_BUFS, name="wd")
                nc.sync.dma_start(w[:], dram.ap()[g])
                wd_lists[p].append(w)

        # ---- queue B (gpsimd, SWDGE): gathered-x / cw / shared-gemm2
        # prefetches, led by a "gate" read of an early shared activation tile
        # (emitted after its producer, so the data dep holds the in-order
        # queue back) - they never race the startup or the weight streams.
        def b_part():
            # gate on the FIRST shared activation tile (ready ~28us); shared
            # t-tiles are 13.7us each, so gating on at[:,1] would hold wsd
            # past the shared-gemm2 start
            nc.gpsimd.dma_start(dum_d.ap()[:], ats[0][0:1, 0, 0:8])
            nc.gpsimd.dma_start(wsd[:], wsd_d.ap()[:])
            nc.gpsimd.dma_start(xgs[0][:], xg_d[0].ap()[:])
            nc.gpsimd.dma_start(cws[0][:], cw_d[0].ap()[:])
            nc.gpsimd.dma_start(xgs[1][:], xg_d[1].ap()[:])
            nc.gpsimd.dma_start(cws[1][:], cw_d[1].ap()[:])
            nc.gpsimd.dma_start(xgs[2][:], xg_d[2].ap()[:])
            nc.gpsimd.dma_start(cws[2][:], cw_d[2].ap()[:])

        def ffn(p, xg, cwB, out_dram, C, it_cnt, chunks,
                mixed_copy=False, og_sz=OG, post_t=None, post_og=None,
                split_last_out=False):
            at = ats[p]

            def act(t, off, n, psg, psu):
                tmp = sb.tile([128, n], f32, tag="tmp", bufs=2, name="tmp")
                nc.scalar.activation(tmp[:], psg[:], ACT_SILU)
                if cwB is not None:
                    tmp2 = sb.tile([128, n], f32, tag="tmp2", bufs=2, name="tmp2")
                    nc.vector.tensor_mul(tmp2[:], tmp[:], psu[:])
                    nc.vector.tensor_mul(at[:, t, off:off + n], tmp2[:],
                                         cwB[:, off:off + n])
                else:
                    nc.vector.tensor_mul(at[:, t, off:off + n], tmp[:], psu[:])

            # GEMM1 + silu*up*cw -> at (activations^T, [i, tokens], bf16)
            if p == 0:
                # chunk-outer: all three weight tiles sweep chunk-0 columns
                # first (small cold-start head, chunk-1 x^T deadline +20us);
                # weight tiles stay resident so nothing is re-loaded.
                emit_shared_head()
                for ci, (off, n) in enumerate(chunks):
                    for t in range(it_cnt):
                        wgu = wgu_lists[0][t]
                        psg = ps.tile([128, n], f32, tag="psg", bufs=2, name="psg")
                        psu = ps.tile([128, n], f32, tag="psu", bufs=3, name="psu")
                        if ci == 0 and t == 0:
                            # k-pair-interleaved chains: overlap the arrival
                            # of the chunk-0 x^T quarters
                            for kq in range(8):
                                for j, pp in ((0, psg), (1, psu)):
                                    for k in range(kq * 2, kq * 2 + 2):
                                        nc.tensor.matmul(
                                            pp[:], wgu[:, j, k, :],
                                            xg[:, ci, k, 0:n],
                                            start=(k == 0), stop=(k == KT - 1))
                        else:
                            for j, pp in ((0, psg), (1, psu)):
                                for k in range(KT):
                                    nc.tensor.matmul(pp[:], wgu[:, j, k, :],
                                                     xg[:, ci, k, 0:n],
                                                     start=(k == 0),
                                                     stop=(k == KT - 1))
                        act(t, off, n, psg, psu)
                        if ci == 0 and t == 0:
                            b_part()
                # next-phase weight prefetch: emitted only now because the
                # recycled ring slots' readers span BOTH chunk passes
                mk_wgu(1, range(6))
                mk_wd(1, range(6))
            else:
                for t in range(it_cnt):
                    if t >= len(wgu_lists[p]):
                        mk_wgu(p, [t])
                    wgu = wgu_lists[p][t]
                    for off, n in chunks:
                        psg = ps.tile([128, n], f32, tag="psg", bufs=2, name="psg")
                        psu = ps.tile([128, n], f32, tag="psu", bufs=3, name="psu")
                        for j, pp in ((0, psg), (1, psu)):
                            for k in range(KT):
                                nc.tensor.matmul(pp[:], wgu[:, j, k, :],
                                                 xg[:, k, off:off + n],
                                                 start=(k == 0),
                                                 stop=(k == KT - 1))
                        act(t, off, n, psg, psu)
                    if post_t is not None and t in post_t:
                        post_t[t]()
            # GEMM2 (transposed): y^T[d-tile] = sum_k wdT[dt,k].T @ at[k]
            wds = wd_lists[p]
            ci = 0
            for og in range(DT // og_sz):
                ysb = sb.tile([128, og_sz, C], bf16, tag="ysb", bufs=3, name="ysb")
                last = split_last_out and og == DT // og_sz - 1
                for e in range(og_sz):
                    dt = og * og_sz + e
                    wd = wds[dt // 2][:, dt % 2] if wds is not None \
                        else wsd[:, dt]
                    for off, n in chunks:
                        psy = ps.tile([128, n], f32, tag="psy", bufs=3, name="psy")
                        for k in range(it_cnt):
                            nc.tensor.matmul(psy[:], wd[:, k, :],
                                             at[:, k, off:off + n],
                                             start=(k == 0), stop=(k == it_cnt - 1))
                        if mixed_copy and ci % 2:
                            nc.scalar.activation(ysb[:, e, off:off + n], psy[:],
                                                 ACT_COPY)
                        else:
                            nc.vector.tensor_copy(ysb[:, e, off:off + n], psy[:])
                        ci += 1
                    if last and e == og_sz // 2 - 1:
                        # drain the first half early to shorten the kernel tail
                        nc.scalar.dma_start(out_dram.ap()[og][:, :og_sz // 2],
                                            ysb[:, :og_sz // 2])
                if last:
                    nc.scalar.dma_start(out_dram.ap()[og][:, og_sz // 2:],
                                        ysb[:, og_sz // 2:])
                elif p == 0 and og % 2:
                    # the shared phase is output-drain limited: odd groups
                    # drain on the sync queue (idle once the next-phase
                    # weight prefetch is issued) to double drain throughput
                    nc.sync.dma_start(out_dram.ap()[og], ysb[:])
                else:
                    nc.scalar.dma_start(out_dram.ap()[og], ysb[:])
                if post_og is not None and og in post_og:
                    post_og[og]()

        ffn(0, xt, None, ys_d, T, SIT, [(0, 512), (512, 512)],
            mixed_copy=True, og_sz=OGS)
        ffn(1, xgs[0], cws[0], yr_d[0], caps[0], IT, [(0, caps[0])],
            post_og={0: lambda: (mk_wd(1, range(6, 8)),
                                 mk_wgu(2, range(3)), mk_wd(2, range(2)))})
        ffn(2, xgs[1], cws[1], yr_d[1], caps[1], IT, [(0, caps[1])],
            post_t={7: lambda: mk_wd(2, range(2, 4)),
                    8: lambda: mk_wd(2, range(4, 6))},
            post_og={0: lambda: (mk_wd(2, range(6, 8)),
                                 mk_wgu(3, range(3)), mk_wd(3, range(2)))})
        ffn(3, xgs[2], cws[2], yr_d[2], caps[2], IT, [(0, caps[2])],
            post_t={7: lambda: mk_wd(3, range(2, 4)),
                    8: lambda: mk_wd(3, range(4, 6))},
            post_og={0: lambda: mk_wd(3, range(6, 8))},
            split_last_out=True)

    nc.compile()
    return nc


# ----------------------------------------------------------------- kernel
def kernel(x, gate_w, bias, w_gate_up, w_down, shared_w_gate_up,
           shared_w_down, _trace=False):
    x = np.ascontiguousarray(x, dtype=np.float32)
    topk_idx, w = _route(x, gate_w, bias)
    cw_full = w.astype(np.float32) * np.float32(ROUTED_SCALE)

    # expert -> token list + weight list
    toks, wts, counts = [], [], np.zeros(E, dtype=np.int64)
    for e in range(E):
        tsel, ksel = np.where(topk_idx == e)
        toks.append(tsel)
        wts.append(cw_full[tsel, ksel])
        counts[e] = len(tsel)

    caps, uses = _solve_slots(counts)
    S = len(caps)
    # build per-class piece lists: (expert, token_idx_array, weight_array)
    class_pieces = [[] for _ in range(S)]
    for e in range(E):
        pos = 0
        # fill this expert's pieces largest-class first
        for s in range(S):
            for _ in range(uses[e][s]):
                n = min(caps[s], counts[e] - pos)
                class_pieces[s].append((e, toks[e][pos:pos + n], wts[e][pos:pos + n]))
                pos += n
    for s in range(S):
        while len(class_pieces[s]) < NCORES:   # dummy empty pieces
            class_pieces[s].append((0, toks[0][:0], wts[0][:0]))

    # pre-pack each expert's weights once (pieces share the arrays)
    wgu_pack = {}
    wd_pack = {}
    for s in range(S):
        for e, _, _ in class_pieces[s]:
            if e not in wgu_pack:
                wgu_pack[e] = _pack_wgu(w_gate_up[e], IT)
                wd_pack[e] = _pack_wdT(np.ascontiguousarray(w_down[e]), IT, 2)
    xt_arr = _pack_xT(x, T)

    in_maps = []
    for c in range(NCORES):
        m = {}
        for s in range(S):
            e, ptoks, pwts = class_pieces[s][c]
            m[f"xg{s}"] = _pack_xT(x[ptoks], caps[s])
            cwb = np.zeros((128, caps[s]), dtype=np.float32)
            cwb[:, :len(pwts)] = pwts[None, :]
            m[f"cw{s}"] = cwb
            m[f"wgu{s}"] = wgu_pack[e]
            m[f"wd{s}"] = wd_pack[e]
        # shared expert slice (rows [352c, 352c+352), zero-padded to 384)
        gsl = np.zeros((2 * SIP, D), dtype=np.float32)
        gsl[:SSLICE] = shared_w_gate_up[SSLICE * c: SSLICE * (c + 1)]
        gsl[SIP:SIP + SSLICE] = shared_w_gate_up[SI + SSLICE * c: SI + SSLICE * (c + 1)]
        m["wsgu"] = _pack_wgu(gsl, SIT)
        sds = np.zeros((D, SIP), dtype=np.float32)
        sds[:, :SSLICE] = shared_w_down[:, SSLICE * c: SSLICE * (c + 1)]
        # wsd as one partition-major array [128, DT, SIT, 128]
        m["wsd"] = np.ascontiguousarray(
            _pack_wdT(sds, SIT, 1).reshape(DT, 128, SIT, 128)
            .transpose(1, 0, 2, 3))
        m["xt"] = np.stack([xt_arr[:, :, :512], xt_arr[:, :, 512:]], axis=0)
        in_maps.append(m)

    nc = _build(caps)
    kw = {}
    if _trace:
        kw = dict(trace=True, trace_cores=list(range(NCORES)))
    res = run_bass_kernel_spmd(nc, in_maps, core_ids=list(range(NCORES)), **kw)

    y = np.zeros((T, D), dtype=np.float32)
    for c in range(NCORES):
        y += _unpack_y(res.results[c]["ys"], T).T
    for c in range(NCORES):
        for s in range(S):
            e, ptoks, _ = class_pieces[s][c]
            n = len(ptoks)
            if n:
                y[ptoks] += _unpack_y(res.results[c][f"yr{s}"], caps[s])[:, :n].T
    if _trace:
        return y, res
    return y
